# revision 23
# baseline (speedup 1.0000x reference)
"""Multi-head causal self-attention on 8 Trainium2 NeuronCores.

Sharding: core c -> batch b = c // 2, heads 4*(c % 2) .. +4  (data parallel on
B, tensor parallel on heads).  Each core computes its 4 heads' attention for
its batch plus the partial out-projection; the host sums the two partials per
batch and adds b_out.

Per-core layout:
  xT   [D, T]      x[b] transposed on host (bf16)
  qT/kT [128, 2, T] head-major: partitions = 2 heads x 64, 2 m-tiles
  v    [128, 16, 260] natural [T, hd] per head + a ones column (gives the
                    softmax denominator for free during the AV matmul)
  scores are computed transposed: sT[k, q] = kT.T @ q, both heads of a pair
  packed into one [128, 2, 512] PSUM tile so a single evacuation call
  covers both; exp'd during PSUM->SBUF evacuation (no max subtraction:
  |scores| <= ~3 here), causality via tile skipping/trimming + one
  upper-triangular 128x128 mask constant.

Attention runs in 512-query units (qb, head-pair).  The PE instruction
stream is software-pipelined with the AV matmuls lagging the score
matmuls by one key-tile step -- the PE queue is strictly in-order, so
without the lag every AV would stall on its own tile's exp evacuation.
AV accumulator PSUM sets alternate per unit so normalization (which
starts with an immediate PSUM-freeing copy) overlaps the next units.
The exp evacuation alternates by key-tile parity between ACT (exact
spline exp) and DVE (Schraudolph bit-trick exp2 via tensor_scalar ->
int16 -> bf16 bitcast, ~3% per element, cancels in softmax).  The
out-projection packs head pairs into K=128 accumulations (odd heads hop
partitions via SBUF-to-SBUF DMA).
"""

import os
import sys
from contextlib import ExitStack

import numpy as np

for _p in ("/opt/trn_rl_repo", "/opt/pypackages"):
    if os.path.isdir(_p) and _p not in sys.path:
        sys.path.append(_p)

import concourse.bass as bass
from concourse import bacc
import concourse.mybir as mybir
import concourse.tile as tile
from concourse.bass_utils import run_bass_kernel_spmd
from concourse.masks import make_upper_triangular


B, T, D = 4, 2048, 512
H, HD = 8, 64
HPC = 4  # heads per core
P = 128
KT = D // P  # k-tiles over the model dim
QB = 512  # query-unit width / psum bank width
NKT = T // P  # key tiles
NU = T // QB  # query blocks
VW = HD + 1  # v columns per head incl. the ones column

F32 = mybir.dt.float32
I16 = mybir.dt.int16
BF16 = mybir.dt.bfloat16
MMDT = BF16
EXP = mybir.ActivationFunctionType.Exp

# fast-exp constants: exp(s/8) ~= bitcast_bf16(int16(s * FE_A + FE_B))
_LOG2E = 1.4426950408889634
FE_A = _LOG2E * 128.0 / 8.0
FE_B = 127.0 * 128.0 - 5.6

try:
    import ml_dtypes
    _NP_MMDT = np.dtype(ml_dtypes.bfloat16)
except ImportError:
    _NP_MMDT = np.float32


def build_bass():
    nc = bacc.Bacc()
    xT = nc.declare_dram_parameter("xT", [D, T], MMDT, isOutput=False)
    wqa = nc.declare_dram_parameter("wqa", [P, KT, 2 * P], MMDT, isOutput=False)
    wka = nc.declare_dram_parameter("wka", [P, KT, 2 * P], MMDT, isOutput=False)
    # q/k biases, laid out [channel % 128, channel // 128] for ACT bias APs
    wqkb = nc.declare_dram_parameter("wqkb", [P, 4], F32, isOutput=False)
    wva = nc.declare_dram_parameter("wva", [P, KT, HPC * VW], MMDT, isOutput=False)
    wo = nc.declare_dram_parameter("wo", [P, 2, D], MMDT, isOutput=False)
    y = nc.declare_dram_parameter("y", [T, D], BF16, isOutput=True)

    with tile.TileContext(nc) as tc, ExitStack() as ctx:
        consts = ctx.enter_context(tc.tile_pool(name="consts", bufs=1))
        qkv = ctx.enter_context(tc.tile_pool(name="qkv", bufs=1))
        attn = ctx.enter_context(tc.tile_pool(name="attn", bufs=1))
        etp = ctx.enter_context(tc.tile_pool(name="etp", bufs=4))
        nrm = ctx.enter_context(tc.tile_pool(name="nrm", bufs=3))
        yevac = ctx.enter_context(tc.tile_pool(name="yevac", bufs=3))
        # PSUM: "mm" 2 bufs x 4KB/partition (2 banks each) = 4 banks;
        # o{i}{s} 4 x [128,512]f32 (1 bank each) = 4 banks.  AV accumulator
        # sets s alternate per unit; QKV + out-proj borrow "mm".
        mmps = ctx.enter_context(tc.tile_pool(name="mmps", bufs=2, space="PSUM"))
        aps = ctx.enter_context(tc.tile_pool(name="aps", bufs=1, space="PSUM"))

        # ---- inputs / constants into SBUF (ordered for early compute start;
        # issue spread across the three DMA-capable engine queues)
        x_sb = consts.tile([P, KT, T], MMDT)
        wq_sb = consts.tile([P, KT, 2 * P], MMDT)
        wk_sb = consts.tile([P, KT, 2 * P], MMDT)
        wv_sb = consts.tile([P, KT, HPC * VW], MMDT)
        wqkb_sb = consts.tile([P, 4], F32)
        wo_sb = consts.tile([P, 2, D], MMDT)

        nc.sync.dma_start(out=wq_sb, in_=wqa[:])
        nc.scalar.dma_start(out=wk_sb, in_=wka[:])
        for kt in range(KT):
            eng = (nc.sync, nc.scalar, nc.gpsimd, nc.sync)[kt]
            eng.dma_start(
                out=x_sb[:, kt, 0 : T // 2], in_=xT[kt * P : (kt + 1) * P, 0 : T // 2]
            )
        nc.gpsimd.dma_start(out=wqkb_sb, in_=wqkb[:])
        nc.scalar.dma_start(out=wv_sb, in_=wva[:])
        for kt in range(KT):
            eng = (nc.sync, nc.scalar, nc.gpsimd, nc.sync)[kt]
            eng.dma_start(
                out=x_sb[:, kt, T // 2 : T], in_=xT[kt * P : (kt + 1) * P, T // 2 : T]
            )
        nc.gpsimd.dma_start(out=wo_sb, in_=wo[:])

        # triu[k, q] = 1 iff q >= k: allowed region of a diagonal block in
        # transposed-score space.  gpsimd affine_select needs f32; cast after.
        triu_st = consts.tile([P, P], F32)
        make_upper_triangular(nc, triu_st, val=1.0, diag=True)
        triu = consts.tile([P, P], MMDT)
        nc.vector.tensor_copy(triu, triu_st)

        # ---- QKV projections
        qT_sb = qkv.tile([P, 2, T], MMDT)
        kT_sb = qkv.tile([P, 2, T], MMDT)
        v_sb = qkv.tile([P, NKT, HPC * VW], MMDT)

        WB = 1024  # bf16 moving-operand max

        def qk_proj(wi, w_sb, dst, m, nb):
            ps = mmps.tile([P, WB], F32, tag="mm", name="ps")
            for lo in range(0, WB, QB):  # psum-bank-sized writes
                for kt in range(KT):
                    nc.tensor.matmul(
                        ps[:, lo : lo + QB],
                        lhsT=w_sb[:, kt, m * P : (m + 1) * P],
                        rhs=x_sb[:, kt, nb * WB + lo : nb * WB + lo + QB],
                        start=(kt == 0),
                        stop=(kt == KT - 1),
                    )
            nc.scalar.activation(
                out=dst[:, m, nb * WB : (nb + 1) * WB], in_=ps,
                func=mybir.ActivationFunctionType.Identity,
                bias=wqkb_sb[:, 2 * wi + m : 2 * wi + m + 1],
            )

        # v bias is folded into the host-side output bias (b_v @ W_out adds a
        # constant row after softmax-normalize + out-projection), so v here is
        # bias-free; the denominator ones-columns are memset directly.
        def v_proj(tt):
            tag = f"o{tt % 2}{'ab'[(tt // 2) % 2]}"
            ps = aps.tile([P, QB], F32, tag=tag, name="vps")
            for kt in range(KT):
                nc.tensor.matmul(
                    ps[:, 0 : HPC * VW],
                    lhsT=x_sb[:, kt, tt * P : (tt + 1) * P],
                    rhs=wv_sb[:, kt, :],
                    start=(kt == 0),
                    stop=(kt == KT - 1),
                )
            nc.scalar.copy(v_sb[:, tt, :], ps[:, 0 : HPC * VW])
            ones_cols = v_sb[:, tt, :].rearrange("p (h w) -> p h w", w=VW)[:, :, HD]
            nc.gpsimd.memset(ones_cols, 1.0)

        # issue order: everything the first attention units need first
        for m in range(2):
            qk_proj(0, wq_sb, qT_sb, m, 0)
            qk_proj(1, wk_sb, kT_sb, m, 0)
        for tt in range(NKT // 2):
            v_proj(tt)
        for m in range(2):
            qk_proj(0, wq_sb, qT_sb, m, 1)
            qk_proj(1, wk_sb, kT_sb, m, 1)
        for tt in range(NKT // 2, NKT):
            v_proj(tt)

        # ---- attention in 512-query units, PE-stream software-pipelined
        # attn pair tiles: rows 0-63 = even head, 64-127 = odd head of pair
        attn_p = [
            attn.tile([P, T], MMDT, tag=f"attnp{hp}", name=f"attnp{hp}")
            for hp in range(2)
        ]

        # Deferred-emission machinery: ops whose inputs arrive with latency
        # (the normalization multiplies waiting on the GpSimd broadcast, the
        # out-projection waiting on normalized attention) are emitted several
        # PE steps after their producers, so no in-order engine queue ever
        # blocks on an unmet dependency in front of PE-feeding work.
        import heapq
        import itertools

        step_box = [0]
        seq = itertools.count()
        deferred = []  # heap of (due_step, seq, fn)

        def defer(delta, fn):
            heapq.heappush(deferred, (step_box[0] + delta, next(seq), fn))

        def flush():
            while deferred and deferred[0][0] <= step_box[0]:
                heapq.heappop(deferred)[2]()

        def emit_outproj_tt(tt):
            # y[t, d] = attn_p0.T @ wo_p0 + attn_p1.T @ wo_p1  (K=128 each)
            ps = mmps.tile([P, D], F32, tag="mm", name="yps")
            for hp in range(2):
                nc.tensor.matmul(
                    ps,
                    lhsT=attn_p[hp][:, tt * P : (tt + 1) * P],
                    rhs=wo_sb[:, hp, :],
                    start=(hp == 0),
                    stop=(hp == 1),
                )
            yt = yevac.tile([P, D], BF16, tag="yt", name="yt")
            if tt % 2:
                nc.scalar.copy(yt, ps)
            else:
                nc.vector.tensor_copy(yt, ps)
            nc.sync.dma_start(out=y[tt * P : (tt + 1) * P, :], in_=yt)

        def emit_norm(hp, qb, opss):
            # normalize: row HD of ops is the softmax denominator.  The
            # copies evacuate the AV psum immediately (freeing the banks);
            # reciprocals/DMAs/broadcasts chain off them inline (their
            # queues have nothing PE-feeding behind); the multiplies, which
            # wait on the broadcast, are emitted a few steps later.
            cols = slice(qb * QB, (qb + 1) * QB)
            atts, bcs = [], []
            for i in (0, 1):
                att_sb = nrm.tile([VW, QB], F32, tag=f"att{i}", name="att")
                if i == 0:
                    nc.vector.tensor_copy(att_sb, opss[i][0:VW, :])
                else:
                    nc.scalar.copy(att_sb, opss[i][0:VW, :])
                rec = nrm.tile([VW, QB], F32, tag=f"rec{i}", name="rec")
                nc.vector.reciprocal_approx_fast(out=rec, in_=att_sb)
                den0 = nrm.tile([1, QB], F32, tag=f"den{i}", name="den0")
                nc.sync.dma_start(out=den0, in_=rec[HD : HD + 1, :])
                bc = nrm.tile([HD, QB], F32, tag=f"bc{i}", name="bc")
                nc.gpsimd.partition_broadcast(bc, den0)
                atts.append(att_sb)
                bcs.append(bc)

            def s_mul():
                nc.vector.tensor_mul(attn_p[hp][0:HD, cols], atts[0][0:HD, :], bcs[0])
                # odd head: normalize into a scratch at lanes 0-63, then
                # DMA-hop to lanes 64-127 of the pair tile
                odd = nrm.tile([HD, QB], MMDT, tag="odd", name="odd")
                nc.vector.tensor_mul(odd, atts[1][0:HD, :], bcs[1])
                nc.sync.dma_start(out=attn_p[hp][HD:P, cols], in_=odd)

            defer(5, s_mul)
            if hp == 1:
                for k, tt in enumerate(range(qb * (QB // P), (qb + 1) * (QB // P))):
                    defer(9 + k, lambda tt=tt: emit_outproj_tt(tt))

        units = [(qb, hp) for hp in range(HPC // 2) for qb in range(NU)]
        pending = None  # (av_emitter, is_last_kt, norm_emitter)
        for u, (qb, hp) in enumerate(units):
            pair = (2 * hp, 2 * hp + 1)
            qhs = [
                qT_sb[(h % 2) * HD : (h % 2) * HD + HD, h // 2, :] for h in pair
            ]
            khs = [
                kT_sb[(h % 2) * HD : (h % 2) * HD + HD, h // 2, :] for h in pair
            ]
            st = "ab"[u % 2]
            opss = [
                aps.tile([P, QB], F32, tag=f"o{i}{st}", name=f"o{i}{st}")
                for i in range(2)
            ]
            nkt = (qb + 1) * (QB // P)
            for kt in range(nkt):
                off = max(0, kt * P - qb * QB)
                # scores for both heads into one [128, 2, 512] psum tile
                sps = mmps.tile([P, 2, QB], F32, tag="mm", name="sps")
                for i in (0, 1):
                    nc.tensor.matmul(
                        sps[:, i, off:QB],
                        lhsT=khs[i][:, kt * P : (kt + 1) * P],
                        rhs=qhs[i][:, qb * QB + off : (qb + 1) * QB],
                        start=True,
                        stop=True,
                    )
                eT = etp.tile([P, 2, QB], MMDT, tag="eT", name="eT")
                if kt % 7 in (1, 3, 6):  # ~43% of tiles on DVE fast-exp
                    nc.vector.tensor_scalar(
                        out=eT[:, :, off:QB].bitcast(I16),
                        in0=sps[:, :, off:QB],
                        scalar1=FE_A, scalar2=FE_B,
                        op0=mybir.AluOpType.mult,
                        op1=mybir.AluOpType.add,
                    )
                else:
                    nc.scalar.activation(
                        out=eT[:, :, off:QB], in_=sps[:, :, off:QB],
                        func=EXP, scale=1.0 / np.sqrt(HD),
                    )
                if kt * P >= qb * QB:  # diagonal-crossing block
                    for i in (0, 1):
                        nc.vector.tensor_mul(
                            eT[:, i, off : off + P], eT[:, i, off : off + P], triu
                        )

                # emit the PREVIOUS step's AV matmuls now (one-step lag keeps
                # the in-order PE queue from stalling on this tile's exp),
                # then any due deferred work
                if pending is not None:
                    pending[0]()
                    if pending[1]:
                        pending[2]()
                flush()
                step_box[0] += 1

                def av(eT=eT, off=off, kt=kt, opss=opss, pair=pair, nkt=nkt):
                    for i in (0, 1):
                        nc.tensor.matmul(
                            opss[i][0:VW, off:QB],
                            lhsT=v_sb[:, kt, pair[i] * VW : (pair[i] + 1) * VW],
                            rhs=eT[:, i, off:QB],
                            start=(kt == 0),
                            stop=(kt == nkt - 1),
                        )

                def norm(hp=hp, qb=qb, opss=opss):
                    emit_norm(hp, qb, opss)

                pending = (av, kt == nkt - 1, norm)
        pending[0]()
        pending[2]()
        while deferred:
            heapq.heappop(deferred)[2]()

    nc.compile()
    return nc


def make_in_maps(x, W_qkv, b_qkv, W_out):
    x = np.asarray(x, np.float32)
    W_qkv = np.asarray(W_qkv, np.float32)
    b_qkv = np.asarray(b_qkv, np.float32)
    W_out = np.asarray(W_out, np.float32)
    in_maps = []
    for c in range(2 * B):
        b, g = divmod(c, 2)
        ch = g * HPC * HD
        wqa = W_qkv[:, ch : ch + 256].reshape(KT, P, 2 * P).transpose(1, 0, 2)
        wka = W_qkv[:, D + ch : D + ch + 256].reshape(KT, P, 2 * P).transpose(1, 0, 2)
        wqkb = np.concatenate(
            [
                b_qkv[ch : ch + 256].reshape(2, P).T,
                b_qkv[D + ch : D + ch + 256].reshape(2, P).T,
            ],
            axis=1,
        )  # [128, 4]: cols = q-m0, q-m1, k-m0, k-m1
        wva = np.zeros((D, HPC * VW), np.float32)
        wva3 = wva.reshape(D, HPC, VW)
        wva3[:, :, :HD] = W_qkv[:, 2 * D + ch : 2 * D + ch + 256].reshape(D, HPC, HD)
        wva = wva.reshape(KT, P, HPC * VW).transpose(1, 0, 2)
        wo = W_out[ch : ch + 256, :].reshape(2, P, D).transpose(1, 0, 2)
        in_maps.append(
            {
                "xT": np.ascontiguousarray(x[b].T).astype(_NP_MMDT),
                "wqa": np.ascontiguousarray(wqa).astype(_NP_MMDT),
                "wka": np.ascontiguousarray(wka).astype(_NP_MMDT),
                "wva": np.ascontiguousarray(wva).astype(_NP_MMDT),
                "wqkb": np.ascontiguousarray(wqkb, np.float32),
                "wo": np.ascontiguousarray(wo).astype(_NP_MMDT),
            }
        )
    return in_maps


def assemble(results, b_out, vbias_y):
    b_out = np.asarray(b_out, np.float32) + vbias_y
    out = np.empty((B, T, D), np.float32)
    for b in range(B):
        out[b] = (
            results[2 * b]["y"].astype(np.float32)
            + results[2 * b + 1]["y"].astype(np.float32)
            + b_out[None, :]
        )
    return out


_CACHE = {}


def kernel(x, W_qkv, b_qkv, W_out, b_out):
    if "nc" not in _CACHE:
        _CACHE["nc"] = build_bass()
    in_maps = make_in_maps(x, W_qkv, b_qkv, W_out)
    # v-bias contribution: softmax weights sum to 1, so b_v passes through
    # attention unchanged and lands as (b_v @ W_out) on every token.
    vbias_y = np.asarray(b_qkv, np.float32)[2 * D :] @ np.asarray(W_out, np.float32)
    res = run_bass_kernel_spmd(_CACHE["nc"], in_maps, list(range(2 * B)))
    return assemble(res.results, b_out, vbias_y)
